# revision 1
# baseline (speedup 1.0000x reference)
"""FlowNet correlation kernel for Trainium2 (8 NeuronCores, batch-parallel).

Problem: out[b, d, y, x] = (1/C) * sum_c i1[b,c,y,x] * pad(i2)[b,c,y+dy,x+dx]
  B=8, C=256, H=48, W=64, pad=20, displacements dy,dx in {-20..20 step 2}
  (21x21 = 441), output [8, 441, 48, 64] fp32.

Strategy (per core, one batch element):
  Displacement stride 2 => the problem splits into 4 independent polyphase
  subproblems (y-parity sy, x-parity sx), each a dense +-10 correlation on a
  24x32 quarter image. For each subproblem and each block of 4 sub-rows
  (M = 4*32 = 128 output pixels), compute the all-pairs band via fp32
  matmuls: stationary = i1 block [C, 128], streaming = the padded-i2 window
  (24 sub-rows x 52 sub-cols = 1248 cols, split 468/468/312 to respect the
  512-fp32 PSUM bank limit), accumulating over the two 128-channel k-tiles.
  Scale by 1/C during the PSUM->SBUF copy, then extract the 441 per-pixel
  correlation values with diagonal-access-pattern DMAs (flat SBUF addressing
  couples partition and byte strides) writing directly to HBM in
  [y, x, d] layout (1764-byte contiguous runs). Host transposes to [d, y, x].
"""

import numpy as np

C = 256
H, W = 48, 64
ND = 21          # displacements per axis
D = ND * ND      # 441
SUB_H, SUB_W = H // 2, W // 2      # 24, 32
HP, WP = H + 40, W + 40            # padded full-res 88, 104
BAND_W = 52                        # padded sub-cols
BAND_ROWS = 24                     # window sub-rows per block
BAND_N = BAND_ROWS * BAND_W        # 1248
ROW_SPLITS = [(0, 9), (9, 18), (18, 24)]   # window-row ranges per PSUM bank
N_BLOCKS = SUB_H // 4              # 6

_CACHE = {}


def _build():
    import concourse.bacc as bacc
    import concourse.mybir as mybir
    from concourse.bass_types import AP, SBTensorHandle
    from concourse.tile import TileContext

    f32 = mybir.dt.float32

    def alias_sbuf(nc, name, shape, dtype, offset, base_partition):
        # SBUF tensor view at a fixed byte offset and nonzero base partition.
        # Mirrors alloc_sbuf_tensor_at but rebases the partition origin so
        # diagonal gather APs keep their flat offset inside one partition row
        # (walrus rejects partition-crossing offsets on irregular APs).
        uname = nc._get_name(name, add_next_id=True)
        nc._tensor(uname, list(shape), dtype, type="SB")
        import functools, operator
        per_part = functools.reduce(operator.mul, shape[1:]) * mybir.dt.size(dtype)
        h = SBTensorHandle(
            uname,
            list(shape),
            dtype,
            base_partition=base_partition,
            manual_sbuf_range=(offset, offset + per_part),
            manual_base_name=name,
        )
        mloc = nc.lookup_mloc(h)
        mloc.allocated = True
        mloc.addr = offset
        mloc.base = base_partition
        return h
    nc = bacc.Bacc("TRN2", target_bir_lowering=False, debug=False)
    i1_t = nc.dram_tensor("i1", [C, H, W], f32, kind="ExternalInput")
    i2_t = nc.dram_tensor("i2", [C, H, W], f32, kind="ExternalInput")
    od_t = nc.dram_tensor("od", [H, W, D], f32, kind="ExternalOutput")

    NBUF = 3
    band_full = []
    band_alias = []
    for i in range(NBUF):
        h = nc.alloc_sbuf_tensor(f"bandf{i}", [128, BAND_N], f32)
        addr = nc.lookup_mloc(h).addr
        band_full.append(h)
        band_alias.append(
            [
                alias_sbuf(nc, f"band{i}ry{ry}", [32, BAND_N], f32, addr, 32 * ry)
                for ry in range(4)
            ]
        )

    from bass_rust import add_dep_helper

    last_gathers = [[] for _ in range(NBUF)]

    with TileContext(nc) as tc:
        with (
            tc.tile_pool(name="inp", bufs=1) as inp_pool,
            tc.tile_pool(name="ps", bufs=2, space="PSUM") as ps_pool,
        ):
            i1_sb = [
                inp_pool.tile([128, H * W], f32, name=f"i1k{k}", tag=f"i1k{k}") for k in range(2)
            ]
            i2_sb = [
                inp_pool.tile([128, HP * WP], f32, name=f"i2k{k}", tag=f"i2k{k}") for k in range(2)
            ]
            i1s_sb = [
                [
                    inp_pool.tile(
                        [128, SUB_H * SUB_W], f32, name=f"i1s{k}{s}", tag=f"i1s{k}{s}"
                    )
                    for s in range(4)
                ]
                for k in range(2)
            ]
            i1v = [t[:].rearrange("c (h w) -> c h w", h=H) for t in i1_sb]
            i2v = [t[:].rearrange("c (h w) -> c h w", h=HP) for t in i2_sb]

            for k in range(2):
                cs = slice(128 * k, 128 * (k + 1))
                nc.sync.dma_start(out=i1_sb[k][:], in_=i1_t.ap()[cs])
                v = i2v[k]
                # zero the pad ring (gpsimd; disjoint from the interior DMA)
                nc.gpsimd.memset(v[:, 0:20, :], 0.0)
                nc.gpsimd.memset(v[:, 68:HP, :], 0.0)
                nc.gpsimd.memset(v[:, 20:68, 0:20], 0.0)
                nc.gpsimd.memset(v[:, 20:68, 84:WP], 0.0)
                nc.sync.dma_start(out=v[:, 20:68, 20:84], in_=i2_t.ap()[cs])
                # de-interleave i1 into the 4 polyphase sub-images (gpsimd):
                # stationary matmul operands need a single-stride free dim
                for s in range(4):
                    sy, sx = s >> 1, s & 1
                    nc.gpsimd.tensor_copy(
                        i1s_sb[k][s][:].rearrange(
                            "c (py px) -> c py px", py=SUB_H
                        ),
                        i1v[k][:, sy : sy + 2 * SUB_H - 1 : 2, sx::2],
                    )

            inv_c = 1.0 / C
            for s in range(4):
                sy, sx = s >> 1, s & 1
                for yb in range(N_BLOCKS):
                    Y = 4 * yb
                    ps = ps_pool.tile([128, 1536], f32, name="ps")
                    for j, (r0, r1) in enumerate(ROW_SPLITS):
                        n = (r1 - r0) * BAND_W
                        for k in range(2):
                            lhs = i1s_sb[k][s][:, 32 * Y : 32 * Y + 128]
                            rh = i2v[k][
                                :,
                                2 * (Y + r0) + sy : 2 * (Y + r1 - 1) + sy + 1 : 2,
                                sx::2,
                            ]
                            nc.tensor.matmul(
                                ps[:, 512 * j : 512 * j + n],
                                lhsT=lhs,
                                rhs=rh,
                                start=(k == 0),
                                stop=(k == 1),
                            )
                    bi = (s * N_BLOCKS + yb) % NBUF
                    band = band_full[bi].ap()
                    copies = [
                        nc.vector.tensor_scalar_mul(
                            band[:, 0:468], ps[:, 0:468], inv_c
                        ),
                        nc.vector.tensor_scalar_mul(
                            band[:, 468:936], ps[:, 512:980], inv_c
                        ),
                        nc.scalar.mul(band[:, 936:1248], ps[:, 1024:1336], inv_c),
                    ]
                    # band buffers live outside the tile pools (the gather
                    # aliases rebase partitions, which Tile can't track), so
                    # RAW (gather-after-copy) and WAR (copy-after-gather on
                    # buffer reuse) edges are added explicitly.
                    for c in copies:
                        for g in last_gathers[bi]:
                            add_dep_helper(c.ins, g.ins, reason="band WAR")
                    gathers = []
                    for ry in range(4):
                        rd = AP(
                            band_alias[bi][ry],
                            ry * BAND_W,
                            [[BAND_N + 1, 32], [BAND_W, ND], [1, ND]],
                        )
                        wr = AP(
                            od_t.ap().tensor,
                            (2 * (Y + ry) + sy) * (W * D) + sx * D,
                            [[2 * D, 32], [ND, ND], [1, ND]],
                        )
                        g = nc.sync.dma_start(out=wr, in_=rd)
                        for c in copies:
                            add_dep_helper(g.ins, c.ins, reason="band RAW")
                        gathers.append(g)
                    last_gathers[bi] = gathers

    nc.compile()
    return nc


def _get_program():
    if "nc" not in _CACHE:
        _CACHE["nc"] = _build()
    return _CACHE["nc"]


def kernel(input1: np.ndarray, input2: np.ndarray) -> np.ndarray:
    from concourse import bass_utils

    nc = _get_program()
    input1 = np.ascontiguousarray(input1, dtype=np.float32)
    input2 = np.ascontiguousarray(input2, dtype=np.float32)
    B = input1.shape[0]
    in_maps = [{"i1": input1[b], "i2": input2[b]} for b in range(B)]
    res = bass_utils.run_bass_kernel_spmd(nc, in_maps, core_ids=list(range(B)))
    out = np.stack([r["od"] for r in res.results])  # [B, H, W, D]
    return np.ascontiguousarray(out.transpose(0, 3, 1, 2))  # [B, D, H, W]



# revision 23
# speedup vs baseline: 1.2343x; 1.2343x over previous
"""FlowNet correlation kernel for Trainium2 (8 NeuronCores, batch-parallel).

Problem: out[b, d, y, x] = (1/C) * sum_c i1[b,c,y,x] * pad(i2)[b,c,y+dy,x+dx]
  B=8, C=256, H=48, W=64, pad=20, displacements dy,dx in {-20..20 step 2}
  (21x21 = 441), output [8, 441, 48, 64] fp32.

Strategy (per core, one batch element):
  Displacement stride 2 => 4 polyphase subproblems s=(sy,sx), each a dense
  +-10 correlation on a 24x32 quarter image against a 10-padded i2 sub-image
  [44x52]. Inputs are converted to bf16 on chip (i1 scaled by 1/C during the
  polyphase de-interleave) so matmuls stream at 1 col/cycle.

  Per s, pixel blocks of 8 sub-rows x 16 sub-cols (M=128): all-pairs band
  matmul, stationary = i1 block [128c,128px], moving = the 28x36 i2s window
  restricted to in-bounds rows/cols (out-of-range contributions are exact
  zeros kept in a pre-zeroed band buffer). PSUM [128,1024] split in 2 banks
  (window rows <14 / >=14), packed 26 cols wide.

  Drain PSUM -> band bf16 (vector/scalar, cast on copy). The per-pixel
  displacement extraction is restructured to avoid tiny diagonal DMA
  packets (the old kernel's bottleneck: 64512 x 84B descriptor generation
  serialized ~206us on the Sync queue):
    1. remap: SBUF->SBUF DMA per block moves each pixel's 21 window rows
       (756 contiguous els) to X[48 part = full-res row, 32 chunks], a pure
       partition permutation with 1512B packets.
    2. shear: per output column xg, one vector/scalar copy [48p, 21,21]
       (free-dim offset xg%16) compacts the 441 needed values into C.
    3. one output DMA per x-parity: C [48, 32*441] bf16 -> HBM, 28KB runs.
  Host upcasts bf16->f32 and transposes [sx,y,xg,d] -> [d,y,x].
"""

import numpy as np

C = 256
H, W = 48, 64
ND = 21          # displacements per axis
D = ND * ND      # 441
SUB_H, SUB_W = H // 2, W // 2      # 24, 32
I2S_H, I2S_W = SUB_H + 20, SUB_W + 20   # 44, 52 (pad 10 each side)
RB, XB = 8, 16                     # pixel block: 8 sub-rows x 16 sub-cols
WIN_R, WIN_C = RB + 20, XB + 20    # 28 x 36 window per block
BLK = WIN_R * WIN_C                # 1008 band els per block
N_YB, N_XB = SUB_H // RB, SUB_W // XB   # 3, 2
BAND_N = N_YB * N_XB * BLK         # 6048
XGUARD = (RB - 1) * WIN_C          # 252: guard for sheared chunk starts
XPITCH = XGUARD + SUB_W * BLK      # 32508: X free els
CPITCH = SUB_W * D                 # 14112: C free els

# valid local window rows per yb (i2s rows [10,34) are real data)
ROWR = [(10, 28), (2, 26), (0, 18)]
# valid local window cols per xb (i2s cols [10,42) are real data)
COLR = [(10, 36), (0, 26)]
BANK_SPLIT = 14   # window rows < 14 -> psum bank 0, >= 14 -> bank 1

_CACHE = {}


def _build():
    import concourse.bacc as bacc
    import concourse.mybir as mybir
    from concourse.bass_types import AP, SBTensorHandle
    from concourse.tile import TileContext
    from bass_rust import add_dep_helper

    f32 = mybir.dt.float32
    bf16 = mybir.dt.bfloat16

    def alias_sbuf(nc, name, shape, dtype, offset, base_partition):
        # SBUF view at a fixed byte offset and nonzero base partition, so
        # flat remap APs keep their start offset inside one partition row
        # (walrus rejects partition-crossing offsets on irregular APs).
        uname = nc._get_name(name, add_next_id=True)
        nc._tensor(uname, list(shape), dtype, type="SB")
        import functools, operator
        per_part = functools.reduce(operator.mul, shape[1:]) * mybir.dt.size(dtype)
        h = SBTensorHandle(
            uname,
            list(shape),
            dtype,
            base_partition=base_partition,
            manual_sbuf_range=(offset, offset + per_part),
            manual_base_name=name,
        )
        mloc = nc.lookup_mloc(h)
        mloc.allocated = True
        mloc.addr = offset
        mloc.base = base_partition
        return h

    nc = bacc.Bacc("TRN2", target_bir_lowering=False, debug=False)
    i1_t = nc.dram_tensor("i1", [C, H, W], f32, kind="ExternalInput")
    i2_t = nc.dram_tensor("i2", [C, H, W], f32, kind="ExternalInput")
    od_t = nc.dram_tensor("od", [2, H, SUB_W, D], bf16, kind="ExternalOutput")

    # raw SBUF tensors accessed with flat/irregular APs
    # one buffer per (yb, xb) so each always hosts the same valid-region
    # pattern and the one-time pad memset stays valid across subproblems
    band = [nc.alloc_sbuf_tensor(f"band{i}", [128, BLK], bf16) for i in range(6)]
    # X partitions are ordered q = 16*yb + 8*sy + ry (ry consecutive) so the
    # remap's sheared dim-0 stride is partition-step-1 (pitch - WIN_C), the
    # only crossing form the DGE ucode handles exactly. Host un-permutes.
    x_t = nc.alloc_sbuf_tensor("xt", [48, XPITCH], bf16)
    x_addr = nc.lookup_mloc(x_t).addr
    x_alias = {
        bp: alias_sbuf(nc, f"xal{bp}", [8, XPITCH], bf16, x_addr, bp)
        for bp in (0, 8, 16, 24, 32, 40)
    }

    with TileContext(nc) as tc:
        with (
            tc.tile_pool(name="inp", bufs=1) as inp_pool,
            tc.tile_pool(name="ps", bufs=3, space="PSUM") as ps_pool,
        ):
            stage = [
                inp_pool.tile([128, H * W], f32, name=f"st{i}", tag=f"st{i}")
                for i in range(2)
            ]
            i1s = [
                [
                    inp_pool.tile([128, SUB_H * SUB_W], bf16, name=f"i1s{k}{s}",
                                  tag=f"i1s{k}{s}")
                    for s in range(4)
                ]
                for k in range(2)
            ]
            i2s = [
                [
                    inp_pool.tile([128, I2S_H * I2S_W], bf16, name=f"i2s{k}{s}",
                                  tag=f"i2s{k}{s}")
                    for s in range(4)
                ]
                for k in range(2)
            ]
            c_t = [
                inp_pool.tile([48, CPITCH], bf16, name=f"ct{sx}", tag=f"ct{sx}")
                for sx in range(2)
            ]

            # pre-zero the band buffers once; drains only touch valid
            # window cells, so pad cells stay exactly zero for every s.
            for b in band:
                nc.gpsimd.memset(b.ap(), 0.0)

            # ---- input staging: contiguous loads + on-chip polyphase ----
            i1v = [t[:].rearrange("c (h w) -> c h w", h=SUB_H) for t in []]
            inv_c = 1.0 / C
            dei_eng = [nc.vector, nc.scalar]
            for k in range(2):
                cs = slice(128 * k, 128 * (k + 1))
                st = stage[k]
                stv = st[:].rearrange("c (h w) -> c h w", h=H)
                nc.sync.dma_start(out=st[:], in_=i1_t.ap()[cs])
                for s in range(4):
                    sy, sx = s >> 1, s & 1
                    # block-major layout: pixel block (yb,xb) contiguous at
                    # offset (2yb+xb)*128 (matmul stationary needs a single
                    # free dim). One copy per yb strip (3 free dims).
                    dstv = i1s[k][s][:].rearrange(
                        "c (b r x) -> c b r x", b=N_YB * N_XB, r=RB
                    )
                    for yb in range(N_YB):
                        src = AP(
                            stage[k][:].tensor,
                            (16 * yb + sy) * W + sx,
                            [[H * W, 128], [2 * XB, N_XB], [2 * W, RB], [2, XB]],
                        )
                        eng = dei_eng[(s + yb) % 2]
                        if eng is nc.scalar:
                            eng.mul(dstv[:, 2 * yb : 2 * yb + 2], src, inv_c)
                        else:
                            eng.tensor_scalar_mul(
                                dstv[:, 2 * yb : 2 * yb + 2], src, inv_c
                            )
                for s in range(4):
                    nc.gpsimd.memset(i2s[k][s][:], 0.0)
            for k in range(2):
                cs = slice(128 * k, 128 * (k + 1))
                st = stage[k]
                stv = st[:].rearrange("c (h w) -> c h w", h=H)
                nc.sync.dma_start(out=st[:], in_=i2_t.ap()[cs])
                for s in range(4):
                    sy, sx = s >> 1, s & 1
                    dst = i2s[k][s][:].rearrange("c (h w) -> c h w", h=I2S_H)
                    eng = dei_eng[(s + 1) % 2]
                    if eng is nc.scalar:
                        eng.copy(
                            dst[:, 10:10 + SUB_H, 10:10 + SUB_W],
                            stv[:, sy::2, sx::2],
                        )
                    else:
                        eng.tensor_copy(
                            dst[:, 10:10 + SUB_H, 10:10 + SUB_W],
                            stv[:, sy::2, sx::2],
                        )

            i2vv = [
                [i2s[k][s][:].rearrange("c (h w) -> c h w", h=I2S_H) for s in range(4)]
                for k in range(2)
            ]

            # ---- main loop ----
            s_order = [(0, 0), (1, 0), (0, 1), (1, 1)]
            remap_by_sx = {0: [], 1: []}
            extract_by_sx = {0: [], 1: []}
            drain_eng = [nc.vector, nc.scalar]
            dma_eng = [nc.sync, nc.scalar]
            n_drain = 0
            n_remap = 0
            n_blk = 0
            for si, (sy, sx) in enumerate(s_order):
                s = 2 * sy + sx
                for yb in range(N_YB):
                    for xb in range(N_XB):
                        bi = 2 * yb + xb
                        bnd = band[bi]
                        n_blk += 1
                        bndv = bnd.ap().rearrange("p (r c) -> p r c", r=WIN_R)
                        c0, c1 = COLR[xb]
                        ps = ps_pool.tile([128, 1024], mybir.dt.float32, name="ps")
                        for k in range(2):
                            lhs = i1s[k][s][:, 128 * bi : 128 * bi + 128]
                            for bank in range(2):
                                ra, rb_ = ROWR[yb]
                                ra = max(ra, BANK_SPLIT * bank)
                                rb_ = min(rb_, BANK_SPLIT * (bank + 1))
                                if ra >= rb_:
                                    continue
                                rhs = i2vv[k][s][
                                    :,
                                    RB * yb + ra : RB * yb + rb_,
                                    XB * xb + c0 : XB * xb + c1,
                                ]
                                po = 512 * bank + (ra - BANK_SPLIT * bank) * (c1 - c0)
                                nc.tensor.matmul(
                                    ps[:, po : po + (rb_ - ra) * (c1 - c0)],
                                    lhsT=lhs,
                                    rhs=rhs,
                                    start=(k == 0),
                                    stop=(k == 1),
                                )
                        # drain psum -> band (cast to bf16)
                        for bank in range(2):
                            ra, rb_ = ROWR[yb]
                            ra = max(ra, BANK_SPLIT * bank)
                            rb_ = min(rb_, BANK_SPLIT * (bank + 1))
                            if ra >= rb_:
                                continue
                            po = 512 * bank + (ra - BANK_SPLIT * bank) * (c1 - c0)
                            src = ps[:, po : po + (rb_ - ra) * (c1 - c0)].rearrange(
                                "p (r c) -> p r c", r=rb_ - ra
                            )
                            dst = bndv[:, ra:rb_, c0:c1]
                            eng = drain_eng[n_drain % 2]
                            n_drain += 1
                            if eng is nc.scalar:
                                eng.copy(dst, src)
                            else:
                                eng.tensor_copy(dst, src)
                        # remap: band block -> X. Partition permutation with
                        # the per-ry shear folded into the dim-0 stride
                        # (+1 partition, -WIN_C els), so extraction offsets
                        # are partition-uniform.
                        src = AP(bnd, 0, [[BLK, 128], [1, BLK]])
                        dst = AP(
                            x_alias[16 * yb + 8 * sy],
                            XGUARD + xb * XB * BLK,
                            [[XPITCH - WIN_C, RB], [BLK, XB], [1, BLK]],
                        )
                        g = dma_eng[n_remap % 2].dma_start(out=dst, in_=src)
                        n_remap += 1
                        remap_by_sx[sx].append(g)
                        if sx == 1:
                            # X WAR: sx=1 remaps overwrite what sx=0
                            # extracts read (different handles -> manual)
                            for e in extract_by_sx[0]:
                                add_dep_helper(g.ins, e.ins, reason="X WAR")

                if si % 2 == 1:  # both sy of this sx done -> extract
                    cv = c_t[sx][:].rearrange("p (x a b) -> p x a b", x=SUB_W, a=ND)
                    for xg in range(SUB_W):
                        xl = xg % XB
                        eng = drain_eng[xg % 2]
                        src = AP(
                            x_t,
                            XGUARD + xg * BLK + xl,
                            [[XPITCH, 48], [WIN_C, ND], [1, ND]],
                        )
                        dst = cv[:, xg]
                        if eng is nc.scalar:
                            e = eng.copy(dst, src)
                        else:
                            e = eng.tensor_copy(dst, src)
                        extract_by_sx[sx].append(e)
                        for g in remap_by_sx[sx]:
                            add_dep_helper(e.ins, g.ins, reason="X RAW")
                    nc.scalar.dma_start(out=od_t.ap()[sx], in_=c_t[sx][:])

    nc.compile()
    return nc


def _get_program():
    if "nc" not in _CACHE:
        _CACHE["nc"] = _build()
    return _CACHE["nc"]


def kernel(input1: np.ndarray, input2: np.ndarray) -> np.ndarray:
    from concourse import bass_utils

    nc = _get_program()
    input1 = np.ascontiguousarray(input1, dtype=np.float32)
    input2 = np.ascontiguousarray(input2, dtype=np.float32)
    B = input1.shape[0]
    in_maps = [{"i1": input1[b], "i2": input2[b]} for b in range(B)]
    res = bass_utils.run_bass_kernel_spmd(nc, in_maps, core_ids=list(range(B)))
    od = np.stack([np.asarray(r["od"]) for r in res.results])  # [B,2,48,32,441] bf16
    od = od.astype(np.float32)
    # device row order is q = 16*yb + 8*sy + ry; un-permute to y = 2*ys + sy
    q = np.arange(48)
    yf = 16 * (q // 16) + 2 * (q % 8) + (q % 16) // 8
    inv = np.empty(48, dtype=np.int64)
    inv[yf] = q
    od = od[:, :, inv]
    # [b, sx, y, xg, d] -> [b, d, y, xg, sx] -> [b, d, y, x]
    out = od.transpose(0, 4, 2, 3, 1).reshape(B, D, H, W)
    return np.ascontiguousarray(out)


# revision 24
# speedup vs baseline: 1.4131x; 1.1448x over previous
"""FlowNet correlation kernel for Trainium2 (8 NeuronCores, batch-parallel).

Problem: out[b, d, y, x] = (1/C) * sum_c i1[b,c,y,x] * pad(i2)[b,c,y+dy,x+dx]
  B=8, C=256, H=48, W=64, pad=20, displacements dy,dx in {-20..20 step 2}
  (21x21 = 441), output [8, 441, 48, 64] fp32.

Strategy (per core, one batch element):
  Displacement stride 2 => 4 polyphase subproblems s=(sy,sx), each a dense
  +-10 correlation on a 24x32 quarter image. Inputs are staged to the device
  as bf16 (i1 pre-scaled by 1/C, exact in bf16) so matmuls stream at
  1 col/cycle and all on-chip traffic is half-width.

  Per s, pixel blocks of 8 sub-rows x 16 sub-cols (M=128, block-major
  stationary): all-pairs band matmul against the 28x36 window, clipped to
  in-range window rows/cols (out-of-range correlations are exact zeros kept
  in a pre-zeroed band buffer). PSUM [128,1024] in 2 banks (window rows
  </>= 14), packed 26 columns wide. Drain PSUM -> band bf16 (vector/scalar
  copies, cast on copy).

  The per-pixel displacement extraction avoids both tiny diagonal DMA
  packets (the v0 bottleneck: 64512 x 84B descriptors, ~206us serialized on
  one queue) and SBUF->SBUF remaps (v2 bottleneck: restricted to 8 DMA
  engines at ~7.5 GB/s each):
    1. dump: per-s band [128,6048] -> HBM scratch, 12KB packets, full rate.
    2. reload: HBM -> X[48p, 32 chunks x 756] where the DRAM-side AP embeds
       the per-ry window-row clip (dim-0 stride 16*6048+36); the SBUF dst is
       a plain partition-step-1 scatter. X partition order q=16yb+8sy+ry.
    3. shear: per output column xg, one vector/scalar copy [48p,21,21]
       (free offset xg*756 + xg%16) compacts 441 values into C.
    4. one output DMA per x-parity: C [48, 32*441] bf16 -> HBM, 28KB runs.
  Host upcasts bf16->f32, un-permutes q->y, and transposes to [d,y,x].
"""

import numpy as np

C = 256
H, W = 48, 64
ND = 21          # displacements per axis
D = ND * ND      # 441
SUB_H, SUB_W = H // 2, W // 2      # 24, 32
RB, XB = 8, 16                     # pixel block: 8 sub-rows x 16 sub-cols
WIN_R, WIN_C = RB + 20, XB + 20    # 28 x 36 window per block
BLK = WIN_R * WIN_C                # 1008 band els per block
N_YB, N_XB = SUB_H // RB, SUB_W // XB   # 3, 2
BAND_N = N_YB * N_XB * BLK         # 6048
RUN = ND * WIN_C                   # 756: per-pixel clipped window
XPITCH = SUB_W * RUN               # 24192: X free els
CPITCH = SUB_W * D                 # 14112: C free els

# valid local window rows per yb (real i2 sub-rows are window rows 10..33)
ROWR = [(10, 28), (2, 26), (0, 18)]
# valid local window cols per xb (real i2 sub-cols are window cols 10..41)
COLR = [(10, 36), (0, 26)]
BANK_SPLIT = 14   # window rows < 14 -> psum bank 0, >= 14 -> bank 1

_CACHE = {}


def _build():
    import concourse.bacc as bacc
    import concourse.mybir as mybir
    from concourse.bass_types import AP, SBTensorHandle
    from concourse.tile import TileContext
    from bass_rust import add_dep_helper

    bf16 = mybir.dt.bfloat16

    def alias_sbuf(nc, name, shape, dtype, offset, base_partition):
        # SBUF view at a fixed byte offset and nonzero base partition, so
        # scatter APs keep their start offset inside one partition row
        # (walrus rejects partition-crossing offsets on irregular APs).
        uname = nc._get_name(name, add_next_id=True)
        nc._tensor(uname, list(shape), dtype, type="SB")
        import functools, operator
        per_part = functools.reduce(operator.mul, shape[1:]) * mybir.dt.size(dtype)
        h = SBTensorHandle(
            uname,
            list(shape),
            dtype,
            base_partition=base_partition,
            manual_sbuf_range=(offset, offset + per_part),
            manual_base_name=name,
        )
        mloc = nc.lookup_mloc(h)
        mloc.allocated = True
        mloc.addr = offset
        mloc.base = base_partition
        return h

    nc = bacc.Bacc("TRN2", target_bir_lowering=False, debug=False)
    i1_t = nc.dram_tensor("i1", [C, H * W], bf16, kind="ExternalInput")
    i2_t = nc.dram_tensor("i2", [C, H * W], bf16, kind="ExternalInput")
    od_t = nc.dram_tensor("od", [2, 48, CPITCH], bf16, kind="ExternalOutput")
    sc_t = nc.dram_tensor("scr", [2, 128, BAND_N], bf16, kind="Internal")

    # raw SBUF tensors accessed with flat/irregular APs
    band = [nc.alloc_sbuf_tensor(f"band{i}", [128, BAND_N], bf16) for i in range(2)]
    x_t = nc.alloc_sbuf_tensor("xt", [48, XPITCH], bf16)
    x_addr = nc.lookup_mloc(x_t).addr
    x_alias = {
        bp: alias_sbuf(nc, f"xal{bp}", [8, XPITCH], bf16, x_addr, bp)
        for bp in (0, 8, 16, 24, 32, 40)
    }

    with TileContext(nc) as tc:
        with (
            tc.tile_pool(name="inp", bufs=1) as inp_pool,
            tc.tile_pool(name="ps", bufs=3, space="PSUM") as ps_pool,
        ):
            stage = [
                inp_pool.tile([128, H * W], bf16, name=f"st{i}", tag=f"st{i}")
                for i in range(2)
            ]
            i1s = [
                [
                    inp_pool.tile([128, SUB_H * SUB_W], bf16, name=f"i1s{k}{s}",
                                  tag=f"i1s{k}{s}")
                    for s in range(4)
                ]
                for k in range(2)
            ]
            i2s = [
                [
                    inp_pool.tile([128, SUB_H * SUB_W], bf16, name=f"i2s{k}{s}",
                                  tag=f"i2s{k}{s}")
                    for s in range(4)
                ]
                for k in range(2)
            ]
            c_t = [
                inp_pool.tile([48, CPITCH], bf16, name=f"ct{sx}", tag=f"ct{sx}")
                for sx in range(2)
            ]

            # pre-zero the band buffers once; drains only touch valid
            # window cells, so pad cells stay exactly zero for every s.
            nc.gpsimd.memset(band[0].ap(), 0.0)
            nc.gpsimd.memset(band[1].ap(), 0.0)

            dei_eng = [nc.vector, nc.scalar]
            n_dei = 0

            def copy_op(eng, dst, src):
                if eng is nc.scalar:
                    return eng.copy(dst, src)
                return eng.tensor_copy(dst, src)

            # ---- input staging: contiguous bf16 loads + on-chip polyphase
            for k in range(2):
                cs = slice(128 * k, 128 * (k + 1))
                nc.sync.dma_start(out=stage[k][:], in_=i1_t.ap()[cs])
                for s in range(4):
                    sy, sx = s >> 1, s & 1
                    # block-major i1s: pixel block (yb,xb) contiguous at
                    # offset (2yb+xb)*128 (stationary needs 1 free dim)
                    dstv = i1s[k][s][:].rearrange(
                        "c (b r x) -> c b r x", b=N_YB * N_XB, r=RB
                    )
                    for yb in range(N_YB):
                        src = AP(
                            stage[k][:].tensor,
                            (16 * yb + sy) * W + sx,
                            [[H * W, 128], [2 * XB, N_XB], [2 * W, RB], [2, XB]],
                        )
                        eng = dei_eng[n_dei % 2]
                        n_dei += 1
                        copy_op(eng, dstv[:, 2 * yb : 2 * yb + 2], src)
            for k in range(2):
                cs = slice(128 * k, 128 * (k + 1))
                nc.sync.dma_start(out=stage[k][:], in_=i2_t.ap()[cs])
                stv = stage[k][:].rearrange("c (h w) -> c h w", h=H)
                for s in range(4):
                    sy, sx = s >> 1, s & 1
                    dst = i2s[k][s][:].rearrange("c (h w) -> c h w", h=SUB_H)
                    eng = dei_eng[n_dei % 2]
                    n_dei += 1
                    copy_op(eng, dst, stv[:, sy::2, sx::2])

            i2vv = [
                [i2s[k][s][:].rearrange("c (h w) -> c h w", h=SUB_H) for s in range(4)]
                for k in range(2)
            ]

            # ---- main loop ----
            s_order = [(0, 0), (1, 0), (0, 1), (1, 1)]
            reload_by_sx = {0: [], 1: []}
            extract_by_sx = {0: [], 1: []}
            drain_eng = [nc.vector, nc.scalar]
            dma_eng = [nc.sync, nc.scalar]
            n_drain = 0
            n_reload = 0
            for si, (sy, sx) in enumerate(s_order):
                s = 2 * sy + sx
                bnd = band[si % 2]
                bndv = bnd.ap().rearrange(
                    "p (b r c) -> p b r c", b=N_YB * N_XB, r=WIN_R
                )
                for yb in range(N_YB):
                    for xb in range(N_XB):
                        bi = 2 * yb + xb
                        c0, c1 = COLR[xb]
                        nco = c1 - c0
                        ps = ps_pool.tile([128, 1024], mybir.dt.float32, name="ps")
                        for k in range(2):
                            lhs = i1s[k][s][:, 128 * bi : 128 * bi + 128]
                            for bank in range(2):
                                ra, rb_ = ROWR[yb]
                                ra = max(ra, BANK_SPLIT * bank)
                                rb_ = min(rb_, BANK_SPLIT * (bank + 1))
                                if ra >= rb_:
                                    continue
                                rhs = i2vv[k][s][
                                    :,
                                    RB * yb + ra - 10 : RB * yb + rb_ - 10,
                                    XB * xb + c0 - 10 : XB * xb + c1 - 10,
                                ]
                                po = 512 * bank + (ra - BANK_SPLIT * bank) * nco
                                nc.tensor.matmul(
                                    ps[:, po : po + (rb_ - ra) * nco],
                                    lhsT=lhs,
                                    rhs=rhs,
                                    start=(k == 0),
                                    stop=(k == 1),
                                )
                        # drain psum -> band (cast to bf16)
                        for bank in range(2):
                            ra, rb_ = ROWR[yb]
                            ra = max(ra, BANK_SPLIT * bank)
                            rb_ = min(rb_, BANK_SPLIT * (bank + 1))
                            if ra >= rb_:
                                continue
                            po = 512 * bank + (ra - BANK_SPLIT * bank) * nco
                            src = ps[:, po : po + (rb_ - ra) * nco].rearrange(
                                "p (r c) -> p r c", r=rb_ - ra
                            )
                            eng = drain_eng[n_drain % 2]
                            n_drain += 1
                            copy_op(eng, bndv[:, bi, ra:rb_, c0:c1], src)

                # dump the whole per-s band to HBM scratch (12KB packets)
                nc.sync.dma_start(out=sc_t.ap()[si % 2], in_=bnd.ap())
                # reload into X: DRAM-side AP clips each pixel to its 21
                # window rows (dim-0 stride embeds the +WIN_C per-ry shift);
                # SBUF side is a plain partition-step-1 scatter.
                for yb in range(N_YB):
                    for xb in range(N_XB):
                        bi = 2 * yb + xb
                        src = AP(
                            sc_t,
                            (si % 2) * 128 * BAND_N + bi * BLK,
                            [[16 * BAND_N + WIN_C, RB], [BAND_N, XB], [1, RUN]],
                        )
                        dst = AP(
                            x_alias[16 * yb + 8 * sy],
                            xb * XB * RUN,
                            [[XPITCH, RB], [RUN, XB], [1, RUN]],
                        )
                        g = dma_eng[n_reload % 2].dma_start(out=dst, in_=src)
                        n_reload += 1
                        reload_by_sx[sx].append(g)
                        if sx == 1:
                            # X WAR: sx=1 reloads overwrite what sx=0
                            # extracts read (different handles -> manual)
                            for e in extract_by_sx[0]:
                                add_dep_helper(g.ins, e.ins, reason="X WAR")

                if si % 2 == 1:  # both sy of this sx done -> extract
                    cv = c_t[sx][:].rearrange("p (x a b) -> p x a b", x=SUB_W, a=ND)
                    for xg in range(SUB_W):
                        xl = xg % XB
                        eng = drain_eng[xg % 2]
                        src = AP(
                            x_t,
                            xg * RUN + xl,
                            [[XPITCH, 48], [WIN_C, ND], [1, ND]],
                        )
                        e = copy_op(eng, cv[:, xg], src)
                        extract_by_sx[sx].append(e)
                        for g in reload_by_sx[sx]:
                            add_dep_helper(e.ins, g.ins, reason="X RAW")
                    nc.sync.dma_start(out=od_t.ap()[sx], in_=c_t[sx][:])

    nc.compile()
    return nc


def _get_program():
    if "nc" not in _CACHE:
        _CACHE["nc"] = _build()
    return _CACHE["nc"]


def kernel(input1: np.ndarray, input2: np.ndarray) -> np.ndarray:
    import ml_dtypes
    from concourse import bass_utils

    nc = _get_program()
    B = input1.shape[0]
    # stage as bf16; fold the exact power-of-two 1/C scale into i1
    i1b = (np.ascontiguousarray(input1, dtype=np.float32) * (1.0 / C)).astype(
        ml_dtypes.bfloat16
    ).reshape(B, C, H * W)
    i2b = np.ascontiguousarray(input2, dtype=np.float32).astype(
        ml_dtypes.bfloat16
    ).reshape(B, C, H * W)
    in_maps = [{"i1": i1b[b], "i2": i2b[b]} for b in range(B)]
    res = bass_utils.run_bass_kernel_spmd(nc, in_maps, core_ids=list(range(B)))
    od = np.stack([np.asarray(r["od"]) for r in res.results])
    od = od.astype(np.float32).reshape(B, 2, 48, SUB_W, D)
    # device row order is q = 16*yb + 8*sy + ry; un-permute to y = 2*ys + sy
    q = np.arange(48)
    yf = 16 * (q // 16) + 2 * (q % 8) + (q % 16) // 8
    inv = np.empty(48, dtype=np.int64)
    inv[yf] = q
    od = od[:, :, inv]
    # [b, sx, y, xg, d] -> [b, d, y, xg, sx] -> [b, d, y, x]
    out = od.transpose(0, 4, 2, 3, 1).reshape(B, D, H, W)
    return np.ascontiguousarray(out)


# revision 29
# speedup vs baseline: 1.9501x; 1.3800x over previous
"""FlowNet correlation kernel for Trainium2 (8 NeuronCores, batch-parallel).

Problem: out[b, d, y, x] = (1/C) * sum_c i1[b,c,y,x] * pad(i2)[b,c,y+dy,x+dx]
  B=8, C=256, H=48, W=64, pad=20, displacements dy,dx in {-20..20 step 2}
  (21x21 = 441), output [8, 441, 48, 64] fp32.

Strategy (per core, one batch element):
  Displacement stride 2 => 4 polyphase subproblems s=(sy,sx), each a dense
  +-10 correlation on a 24x32 quarter image. Inputs are staged to the device
  as bf16 (i1 pre-scaled by 1/C, exact in bf16) so matmuls stream at
  1 col/cycle and all on-chip traffic is half-width.

  Per s, pixel blocks of 8 sub-rows x 16 sub-cols (M=128, block-major
  stationary): all-pairs band matmul against the 28x36 window, clipped to
  in-range window rows/cols (out-of-range correlations are exact zeros kept
  in a pre-zeroed band buffer). PSUM [128,1024] in 2 banks (window rows
  </>= 14), packed 26 columns wide. Drain PSUM -> band bf16 (vector/scalar
  copies, cast on copy).

  The per-pixel displacement extraction avoids both tiny diagonal DMA
  packets (the v0 bottleneck: 64512 x 84B descriptors, ~206us serialized on
  one queue) and SBUF->SBUF remaps (v2 bottleneck: restricted to 8 DMA
  engines at ~7.5 GB/s each):
    1. dump: per-s band [128,6048] -> HBM scratch, 12KB packets, full rate.
    2. reload: HBM -> X[48p, 32 chunks x 756] where the DRAM-side AP embeds
       the per-ry window-row clip (dim-0 stride 16*6048+36); the SBUF dst is
       a plain partition-step-1 scatter. X partition order q=16yb+8sy+ry.
    3. shear: per output column xg, one vector/scalar copy [48p,21,21]
       (free offset xg*756 + xg%16) compacts 441 values into C.
    4. one output DMA per x-parity: C [48, 32*441] bf16 -> HBM, 28KB runs.
  Host upcasts bf16->f32, un-permutes q->y, and transposes to [d,y,x].
"""

import numpy as np

C = 256
H, W = 48, 64
ND = 21          # displacements per axis
D = ND * ND      # 441
SUB_H, SUB_W = H // 2, W // 2      # 24, 32
RB, XB = 8, 16                     # pixel block: 8 sub-rows x 16 sub-cols
WIN_R, WIN_C = RB + 20, XB + 20    # 28 x 36 window per block
BLK = WIN_R * WIN_C                # 1008 band els per block
N_YB, N_XB = SUB_H // RB, SUB_W // XB   # 3, 2
BAND_N = N_YB * N_XB * BLK         # 6048
RUN = ND * WIN_C                   # 756: per-pixel clipped window
XPITCH = SUB_W * RUN               # 24192: X free els
CPITCH = SUB_W * D                 # 14112: C free els

# valid local window rows per yb (real i2 sub-rows are window rows 10..33)
ROWR = [(10, 28), (2, 26), (0, 18)]
# valid local window cols per xb (real i2 sub-cols are window cols 10..41)
COLR = [(10, 36), (0, 26)]
BANK_SPLIT = 14   # window rows < 14 -> psum bank 0, >= 14 -> bank 1

_CACHE = {}


def _build():
    import concourse.bacc as bacc
    import concourse.mybir as mybir
    from concourse.bass_types import AP, SBTensorHandle
    from concourse.tile import TileContext
    from bass_rust import add_dep_helper

    bf16 = mybir.dt.bfloat16

    def alias_sbuf(nc, name, shape, dtype, offset, base_partition):
        # SBUF view at a fixed byte offset and nonzero base partition, so
        # scatter APs keep their start offset inside one partition row
        # (walrus rejects partition-crossing offsets on irregular APs).
        uname = nc._get_name(name, add_next_id=True)
        nc._tensor(uname, list(shape), dtype, type="SB")
        import functools, operator
        per_part = functools.reduce(operator.mul, shape[1:]) * mybir.dt.size(dtype)
        h = SBTensorHandle(
            uname,
            list(shape),
            dtype,
            base_partition=base_partition,
            manual_sbuf_range=(offset, offset + per_part),
            manual_base_name=name,
        )
        mloc = nc.lookup_mloc(h)
        mloc.allocated = True
        mloc.addr = offset
        mloc.base = base_partition
        return h

    nc = bacc.Bacc("TRN2", target_bir_lowering=False, debug=False)
    i1_t = nc.dram_tensor("i1", [C, H * W], bf16, kind="ExternalInput")
    i2_t = nc.dram_tensor("i2", [C, H * W], bf16, kind="ExternalInput")
    od_t = nc.dram_tensor("od", [2, 48, CPITCH], bf16, kind="ExternalOutput")
    # scratch slots padded to 8*RY_STRIDE so the reload's (sy, ry) pair
    # collapses into one 16-count dim-0 (wider SBUF partition span per DMA
    # => more DMA engines participate)
    RY_STRIDE = 16 * BAND_N + WIN_C          # 96804
    SLOT = 8 * RY_STRIDE                      # 774432 (>= 128*BAND_N)
    sc_t = nc.dram_tensor("scr", [2, 2, SLOT], bf16, kind="Internal")

    # raw SBUF tensors accessed with flat/irregular APs
    band = [nc.alloc_sbuf_tensor(f"band{i}", [128, BAND_N], bf16) for i in range(2)]
    x_t = []
    x_alias = []
    for xi in range(2):
        xt = nc.alloc_sbuf_tensor(f"xt{xi}", [48, XPITCH], bf16)
        x_addr = nc.lookup_mloc(xt).addr
        x_t.append(xt)
        x_alias.append({
            bp: alias_sbuf(nc, f"x{xi}al{bp}", [16, XPITCH], bf16, x_addr, bp)
            for bp in (0, 16, 32)
        })

    with TileContext(nc) as tc:
        with (
            tc.tile_pool(name="inp", bufs=1) as inp_pool,
            tc.tile_pool(name="ps", bufs=3, space="PSUM") as ps_pool,
        ):
            stage = [
                inp_pool.tile([128, H * W], bf16, name=f"st{i}", tag=f"st{i}")
                for i in range(2)
            ]
            i1s = [
                [
                    inp_pool.tile([128, SUB_H * SUB_W], bf16, name=f"i1s{k}{s}",
                                  tag=f"i1s{k}{s}")
                    for s in range(4)
                ]
                for k in range(2)
            ]
            i2s = [
                [
                    inp_pool.tile([128, SUB_H * SUB_W], bf16, name=f"i2s{k}{s}",
                                  tag=f"i2s{k}{s}")
                    for s in range(4)
                ]
                for k in range(2)
            ]
            c_t = [
                inp_pool.tile([48, XB * D], bf16, name=f"ct{i}", tag=f"ct{i}")
                for i in range(2)
            ]

            # pre-zero the band buffers once; drains only touch valid
            # window cells, so pad cells stay exactly zero for every s.
            nc.gpsimd.memset(band[0].ap(), 0.0)
            nc.gpsimd.memset(band[1].ap(), 0.0)

            dei_eng = [nc.vector, nc.scalar]
            n_dei = 0

            def copy_op(eng, dst, src):
                if eng is nc.scalar:
                    return eng.copy(dst, src)
                return eng.tensor_copy(dst, src)

            # ---- input staging: contiguous bf16 loads + on-chip polyphase
            for k in range(2):
                cs = slice(128 * k, 128 * (k + 1))
                dma_load = nc.sync if k == 0 else nc.scalar
                dma_load.dma_start(out=stage[k][:], in_=i1_t.ap()[cs])
                for s in range(4):
                    sy, sx = s >> 1, s & 1
                    # block-major i1s: pixel block (yb,xb) contiguous at
                    # offset (2yb+xb)*128 (stationary needs 1 free dim)
                    dstv = i1s[k][s][:].rearrange(
                        "c (b r x) -> c b r x", b=N_YB * N_XB, r=RB
                    )
                    for yb in range(N_YB):
                        src = AP(
                            stage[k][:].tensor,
                            (16 * yb + sy) * W + sx,
                            [[H * W, 128], [2 * XB, N_XB], [2 * W, RB], [2, XB]],
                        )
                        eng = dei_eng[n_dei % 2]
                        n_dei += 1
                        copy_op(eng, dstv[:, 2 * yb : 2 * yb + 2], src)
            for k in range(2):
                cs = slice(128 * k, 128 * (k + 1))
                dma_load = nc.sync if k == 0 else nc.scalar
                dma_load.dma_start(out=stage[k][:], in_=i2_t.ap()[cs])
                stv = stage[k][:].rearrange("c (h w) -> c h w", h=H)
                for s in range(4):
                    sy, sx = s >> 1, s & 1
                    dst = i2s[k][s][:].rearrange("c (h w) -> c h w", h=SUB_H)
                    eng = dei_eng[n_dei % 2]
                    n_dei += 1
                    copy_op(eng, dst, stv[:, sy::2, sx::2])

            i2vv = [
                [i2s[k][s][:].rearrange("c (h w) -> c h w", h=SUB_H) for s in range(4)]
                for k in range(2)
            ]

            # ---- main loop ----
            s_order = [(0, 0), (1, 0), (0, 1), (1, 1)]
            reload_by_sx = {0: [], 1: []}
            extract_by_sx = {0: [], 1: []}
            drain_eng = [nc.vector, nc.scalar]
            dma_eng = [nc.sync, nc.scalar]
            n_drain = 0
            n_reload = 0
            for si, (sy, sx) in enumerate(s_order):
                s = 2 * sy + sx
                bnd = band[si % 2]
                bndv = bnd.ap().rearrange(
                    "p (b r c) -> p b r c", b=N_YB * N_XB, r=WIN_R
                )
                for yb in range(N_YB):
                    for xb in range(N_XB):
                        bi = 2 * yb + xb
                        c0, c1 = COLR[xb]
                        nco = c1 - c0
                        ps = ps_pool.tile([128, 1024], mybir.dt.float32, name="ps")
                        for k in range(2):
                            lhs = i1s[k][s][:, 128 * bi : 128 * bi + 128]
                            for bank in range(2):
                                ra, rb_ = ROWR[yb]
                                ra = max(ra, BANK_SPLIT * bank)
                                rb_ = min(rb_, BANK_SPLIT * (bank + 1))
                                if ra >= rb_:
                                    continue
                                rhs = i2vv[k][s][
                                    :,
                                    RB * yb + ra - 10 : RB * yb + rb_ - 10,
                                    XB * xb + c0 - 10 : XB * xb + c1 - 10,
                                ]
                                po = 512 * bank + (ra - BANK_SPLIT * bank) * nco
                                nc.tensor.matmul(
                                    ps[:, po : po + (rb_ - ra) * nco],
                                    lhsT=lhs,
                                    rhs=rhs,
                                    start=(k == 0),
                                    stop=(k == 1),
                                )
                        # drain psum -> band (cast to bf16)
                        for bank in range(2):
                            ra, rb_ = ROWR[yb]
                            ra = max(ra, BANK_SPLIT * bank)
                            rb_ = min(rb_, BANK_SPLIT * (bank + 1))
                            if ra >= rb_:
                                continue
                            po = 512 * bank + (ra - BANK_SPLIT * bank) * nco
                            src = ps[:, po : po + (rb_ - ra) * nco].rearrange(
                                "p (r c) -> p r c", r=rb_ - ra
                            )
                            eng = drain_eng[n_drain % 2]
                            n_drain += 1
                            copy_op(eng, bndv[:, bi, ra:rb_, c0:c1], src)

                # dump the whole per-s band to HBM scratch (12KB packets)
                nc.sync.dma_start(
                    out=AP(sc_t, (sx * 2 + sy) * SLOT, [[BAND_N, 128], [1, BAND_N]]),
                    in_=bnd.ap(),
                )

                if si % 2 == 1:
                    # reload into X[sx]: the DRAM-side AP clips each pixel to
                    # its 21 window rows; (sy, ry) collapse into one 16-count
                    # dim (slot pitch = 8*RY_STRIDE) so each DMA writes 16
                    # SBUF partitions. Plain partition-step-1 dst.
                    for yb in range(N_YB):
                        for xb in range(N_XB):
                            bi = 2 * yb + xb
                            src = AP(
                                sc_t,
                                sx * 2 * SLOT + bi * BLK,
                                [[RY_STRIDE, 16], [BAND_N, XB], [1, RUN]],
                            )
                            dst = AP(
                                x_alias[sx][16 * yb],
                                xb * XB * RUN,
                                [[XPITCH, 16], [RUN, XB], [1, RUN]],
                            )
                            g = dma_eng[n_reload % 2].dma_start(out=dst, in_=src)
                            n_reload += 1
                            reload_by_sx[sx].append(g)
                    # extract: per output column, compact 441 of 756
                    for xg in range(SUB_W):
                        xb, xl = xg // XB, xg % XB
                        cv = c_t[xb][:].rearrange("p (x a b) -> p x a b", x=XB, a=ND)
                        eng = drain_eng[xg % 2]
                        src = AP(
                            x_t[sx],
                            xg * RUN + xl,
                            [[XPITCH, 48], [WIN_C, ND], [1, ND]],
                        )
                        e = copy_op(eng, cv[:, xl], src)
                        extract_by_sx[sx].append(e)
                        for g in reload_by_sx[sx]:
                            add_dep_helper(e.ins, g.ins, reason="X RAW")
                        if xl == XB - 1:
                            nc.scalar.dma_start(
                                out=od_t.ap()[sx][:, xb * XB * D : (xb + 1) * XB * D],
                                in_=c_t[xb][:],
                            )

    nc.compile()
    return nc


def _get_program():
    if "nc" not in _CACHE:
        _CACHE["nc"] = _build()
    return _CACHE["nc"]


def kernel(input1: np.ndarray, input2: np.ndarray) -> np.ndarray:
    import ml_dtypes
    from concourse import bass_utils

    nc = _get_program()
    B = input1.shape[0]
    # stage as bf16; fold the exact power-of-two 1/C scale into i1
    i1b = (np.ascontiguousarray(input1, dtype=np.float32) * (1.0 / C)).astype(
        ml_dtypes.bfloat16
    ).reshape(B, C, H * W)
    i2b = np.ascontiguousarray(input2, dtype=np.float32).astype(
        ml_dtypes.bfloat16
    ).reshape(B, C, H * W)
    in_maps = [{"i1": i1b[b], "i2": i2b[b]} for b in range(B)]
    res = bass_utils.run_bass_kernel_spmd(nc, in_maps, core_ids=list(range(B)))
    od = np.stack([np.asarray(r["od"]) for r in res.results])
    od = od.astype(np.float32).reshape(B, 2, 48, SUB_W, D)
    # device row order is q = 16*yb + 8*sy + ry; un-permute to y = 2*ys + sy
    q = np.arange(48)
    yf = 16 * (q // 16) + 2 * (q % 8) + (q % 16) // 8
    inv = np.empty(48, dtype=np.int64)
    inv[yf] = q
    od = od[:, :, inv]
    # [b, sx, y, xg, d] -> [b, d, y, xg, sx] -> [b, d, y, x]
    out = od.transpose(0, 4, 2, 3, 1).reshape(B, D, H, W)
    return np.ascontiguousarray(out)


# revision 31
# speedup vs baseline: 2.2709x; 1.1645x over previous
"""FlowNet correlation kernel for Trainium2 (8 NeuronCores, batch-parallel).

Problem: out[b, d, y, x] = (1/C) * sum_c i1[b,c,y,x] * pad(i2)[b,c,y+dy,x+dx]
  B=8, C=256, H=48, W=64, pad=20, displacements dy,dx in {-20..20 step 2}
  (21x21 = 441), output [8, 441, 48, 64] fp32.

Strategy (per core, one batch element):
  Displacement stride 2 => 4 polyphase subproblems s=(sy,sx), each a dense
  +-10 correlation on a 24x32 quarter image. Inputs are staged to the device
  as bf16 (i1 pre-scaled by 1/C, exact in bf16) so matmuls stream at
  1 col/cycle and all on-chip traffic is half-width.

  Per s, pixel blocks of 8 sub-rows x 16 sub-cols (M=128, block-major
  stationary): all-pairs band matmul against the 28x36 window, clipped to
  in-range window rows/cols (out-of-range correlations are exact zeros kept
  in a pre-zeroed band buffer). PSUM [128,1024] in 2 banks (window rows
  </>= 14), packed 26 columns wide. Drain PSUM -> band bf16 (vector/scalar
  copies, cast on copy).

  The per-pixel displacement extraction avoids both tiny diagonal DMA
  packets (the v0 bottleneck: 64512 x 84B descriptors, ~206us serialized on
  one queue) and SBUF->SBUF remaps (v2 bottleneck: restricted to 8 DMA
  engines at ~7.5 GB/s each):
    1. dump: per-s band [128,6048] -> HBM scratch, 12KB packets, full rate.
    2. reload: HBM -> X[48p, 32 chunks x 756] where the DRAM-side AP embeds
       the per-ry window-row clip (dim-0 stride 16*6048+36); the SBUF dst is
       a plain partition-step-1 scatter. X partition order q=16yb+8sy+ry.
    3. shear: per output column xg, one vector/scalar copy [48p,21,21]
       (free offset xg*756 + xg%16) compacts 441 values into C.
    4. one output DMA per x-parity: C [48, 32*441] bf16 -> HBM, 28KB runs.
  Host upcasts bf16->f32, un-permutes q->y, and transposes to [d,y,x].
"""

import numpy as np

C = 256
H, W = 48, 64
ND = 21          # displacements per axis
D = ND * ND      # 441
SUB_H, SUB_W = H // 2, W // 2      # 24, 32
RB, XB = 8, 16                     # pixel block: 8 sub-rows x 16 sub-cols
WIN_R, WIN_C = RB + 20, XB + 20    # 28 x 36 window per block
BLK = WIN_R * WIN_C                # 1008 band els per block
N_YB, N_XB = SUB_H // RB, SUB_W // XB   # 3, 2
BAND_N = N_YB * N_XB * BLK         # 6048
RUN = ND * WIN_C                   # 756: per-pixel clipped window
XPITCH = SUB_W * RUN               # 24192: X free els
CPITCH = SUB_W * D                 # 14112: C free els

# valid local window rows per yb (real i2 sub-rows are window rows 10..33)
ROWR = [(10, 28), (2, 26), (0, 18)]
# valid local window cols per xb (real i2 sub-cols are window cols 10..41)
COLR = [(10, 36), (0, 26)]
BANK_SPLIT = 14   # window rows < 14 -> psum bank 0, >= 14 -> bank 1

_CACHE = {}


def _build():
    import concourse.bacc as bacc
    import concourse.mybir as mybir
    from concourse.bass_types import AP, SBTensorHandle
    from concourse.tile import TileContext
    from bass_rust import add_dep_helper

    bf16 = mybir.dt.bfloat16

    def alias_sbuf(nc, name, shape, dtype, offset, base_partition):
        # SBUF view at a fixed byte offset and nonzero base partition, so
        # scatter APs keep their start offset inside one partition row
        # (walrus rejects partition-crossing offsets on irregular APs).
        uname = nc._get_name(name, add_next_id=True)
        nc._tensor(uname, list(shape), dtype, type="SB")
        import functools, operator
        per_part = functools.reduce(operator.mul, shape[1:]) * mybir.dt.size(dtype)
        h = SBTensorHandle(
            uname,
            list(shape),
            dtype,
            base_partition=base_partition,
            manual_sbuf_range=(offset, offset + per_part),
            manual_base_name=name,
        )
        mloc = nc.lookup_mloc(h)
        mloc.allocated = True
        mloc.addr = offset
        mloc.base = base_partition
        return h

    nc = bacc.Bacc("TRN2", target_bir_lowering=False, debug=False)
    i1_t = nc.dram_tensor("i1", [C, H * W], bf16, kind="ExternalInput")
    i2_t = nc.dram_tensor("i2", [C, H * W], bf16, kind="ExternalInput")
    od_t = nc.dram_tensor("od", [2, 48, CPITCH], bf16, kind="ExternalOutput")
    # scratch slots padded to 8*RY_STRIDE so the reload's (sy, ry) pair
    # collapses into one 16-count dim-0 (wider SBUF partition span per DMA
    # => more DMA engines participate)
    RY_STRIDE = 16 * BAND_N + WIN_C          # 96804
    SLOT = 8 * RY_STRIDE                      # 774432 (>= 128*BAND_N)
    sc_t = nc.dram_tensor("scr", [2, 2, SLOT], bf16, kind="Internal")

    # raw SBUF tensors accessed with flat/irregular APs
    band = [nc.alloc_sbuf_tensor(f"band{i}", [128, BAND_N], bf16) for i in range(2)]
    x_t = []
    x_alias = []
    for xi in range(2):
        xt = nc.alloc_sbuf_tensor(f"xt{xi}", [48, XPITCH], bf16)
        x_addr = nc.lookup_mloc(xt).addr
        x_t.append(xt)
        x_alias.append({
            bp: alias_sbuf(nc, f"x{xi}al{bp}", [16, XPITCH], bf16, x_addr, bp)
            for bp in (0, 16, 32)
        })

    with TileContext(nc) as tc:
        with (
            tc.tile_pool(name="inp", bufs=1) as inp_pool,
            tc.tile_pool(name="ps", bufs=3, space="PSUM") as ps_pool,
        ):
            stage = [
                inp_pool.tile([128, H * W], bf16, name=f"st{i}", tag=f"st{i}")
                for i in range(2)
            ]
            i1s = [
                [
                    inp_pool.tile([128, SUB_H * SUB_W], bf16, name=f"i1s{k}{s}",
                                  tag=f"i1s{k}{s}")
                    for s in range(4)
                ]
                for k in range(2)
            ]
            i2s = [
                [
                    inp_pool.tile([128, SUB_H * SUB_W], bf16, name=f"i2s{k}{s}",
                                  tag=f"i2s{k}{s}")
                    for s in range(4)
                ]
                for k in range(2)
            ]
            c_t = [
                inp_pool.tile([48, XB * D], bf16, name=f"ct{i}", tag=f"ct{i}")
                for i in range(2)
            ]

            # pre-zero the band buffers once; drains only touch valid
            # window cells, so pad cells stay exactly zero for every s.
            nc.gpsimd.memset(band[0].ap(), 0.0)
            nc.gpsimd.memset(band[1].ap(), 0.0)

            dei_eng = [nc.vector, nc.scalar]
            n_dei = 0

            def copy_op(eng, dst, src):
                if eng is nc.scalar:
                    return eng.copy(dst, src)
                return eng.tensor_copy(dst, src)

            # ---- input staging: contiguous bf16 loads + on-chip polyphase
            for k in range(2):
                cs = slice(128 * k, 128 * (k + 1))
                dma_load = nc.sync if k == 0 else nc.scalar
                dma_load.dma_start(out=stage[k][:], in_=i1_t.ap()[cs])
                for s in range(4):
                    sy, sx = s >> 1, s & 1
                    # block-major i1s: pixel block (yb,xb) contiguous at
                    # offset (2yb+xb)*128 (stationary needs 1 free dim)
                    dstv = i1s[k][s][:].rearrange(
                        "c (b r x) -> c b r x", b=N_YB * N_XB, r=RB
                    )
                    for yb in range(N_YB):
                        src = AP(
                            stage[k][:].tensor,
                            (16 * yb + sy) * W + sx,
                            [[H * W, 128], [2 * XB, N_XB], [2 * W, RB], [2, XB]],
                        )
                        eng = dei_eng[n_dei % 2]
                        n_dei += 1
                        copy_op(eng, dstv[:, 2 * yb : 2 * yb + 2], src)
            for k in range(2):
                cs = slice(128 * k, 128 * (k + 1))
                dma_load = nc.sync if k == 0 else nc.scalar
                dma_load.dma_start(out=stage[k][:], in_=i2_t.ap()[cs])
                stv = stage[k][:].rearrange("c (h w) -> c h w", h=H)
                for s in range(4):
                    sy, sx = s >> 1, s & 1
                    dst = i2s[k][s][:].rearrange("c (h w) -> c h w", h=SUB_H)
                    eng = dei_eng[n_dei % 2]
                    n_dei += 1
                    copy_op(eng, dst, stv[:, sy::2, sx::2])

            i2vv = [
                [i2s[k][s][:].rearrange("c (h w) -> c h w", h=SUB_H) for s in range(4)]
                for k in range(2)
            ]

            # ---- main loop ----
            s_order = [(0, 0), (1, 0), (0, 1), (1, 1)]
            reload_by_sx = {(sx, xb): [] for sx in range(2) for xb in range(2)}
            extract_by_sx = {0: [], 1: []}
            drain_eng = [nc.vector, nc.scalar]
            dma_eng = [nc.sync, nc.scalar]
            n_drain = 0
            n_reload = 0
            for si, (sy, sx) in enumerate(s_order):
                s = 2 * sy + sx
                bnd = band[si % 2]
                bndv = bnd.ap().rearrange(
                    "p (b r c) -> p b r c", b=N_YB * N_XB, r=WIN_R
                )
                for yb in range(N_YB):
                    for xb in range(N_XB):
                        bi = 2 * yb + xb
                        c0, c1 = COLR[xb]
                        nco = c1 - c0
                        ps = ps_pool.tile([128, 1024], mybir.dt.float32, name="ps")
                        for k in range(2):
                            lhs = i1s[k][s][:, 128 * bi : 128 * bi + 128]
                            for bank in range(2):
                                ra, rb_ = ROWR[yb]
                                ra = max(ra, BANK_SPLIT * bank)
                                rb_ = min(rb_, BANK_SPLIT * (bank + 1))
                                if ra >= rb_:
                                    continue
                                rhs = i2vv[k][s][
                                    :,
                                    RB * yb + ra - 10 : RB * yb + rb_ - 10,
                                    XB * xb + c0 - 10 : XB * xb + c1 - 10,
                                ]
                                po = 512 * bank + (ra - BANK_SPLIT * bank) * nco
                                nc.tensor.matmul(
                                    ps[:, po : po + (rb_ - ra) * nco],
                                    lhsT=lhs,
                                    rhs=rhs,
                                    start=(k == 0),
                                    stop=(k == 1),
                                )
                        # drain psum -> band (cast to bf16)
                        for bank in range(2):
                            ra, rb_ = ROWR[yb]
                            ra = max(ra, BANK_SPLIT * bank)
                            rb_ = min(rb_, BANK_SPLIT * (bank + 1))
                            if ra >= rb_:
                                continue
                            po = 512 * bank + (ra - BANK_SPLIT * bank) * nco
                            src = ps[:, po : po + (rb_ - ra) * nco].rearrange(
                                "p (r c) -> p r c", r=rb_ - ra
                            )
                            eng = drain_eng[n_drain % 2]
                            n_drain += 1
                            copy_op(eng, bndv[:, bi, ra:rb_, c0:c1], src)

                    # dump this yb's two blocks to HBM scratch right after
                    # their drains (pipelines the reload into the compute)
                    nc.sync.dma_start(
                        out=AP(
                            sc_t,
                            (sx * 2 + sy) * SLOT + 2 * yb * BLK,
                            [[BAND_N, 128], [1, 2 * BLK]],
                        ),
                        in_=AP(bnd, 2 * yb * BLK, [[BAND_N, 128], [1, 2 * BLK]]),
                    )
                    if si % 2 == 1:
                        # both sy of this (sx, yb) dumped -> reload into
                        # X[sx]. The DRAM-side AP clips each pixel to its 21
                        # window rows; (sy, ry) collapse into one 16-count
                        # dim (slot pitch = 8*RY_STRIDE) so each DMA writes
                        # 16 SBUF partitions. Plain partition-step-1 dst.
                        for xb in range(N_XB):
                            bi = 2 * yb + xb
                            src = AP(
                                sc_t,
                                sx * 2 * SLOT + bi * BLK,
                                [[RY_STRIDE, 16], [BAND_N, XB], [1, RUN]],
                            )
                            dst = AP(
                                x_alias[sx][16 * yb],
                                xb * XB * RUN,
                                [[XPITCH, 16], [RUN, XB], [1, RUN]],
                            )
                            g = dma_eng[n_reload % 2].dma_start(out=dst, in_=src)
                            n_reload += 1
                            reload_by_sx[(sx, xb)].append(g)

                if si % 2 == 1:
                    # extract: per output column, compact 441 of 756
                    for xg in range(SUB_W):
                        xb, xl = xg // XB, xg % XB
                        cv = c_t[xb][:].rearrange("p (x a b) -> p x a b", x=XB, a=ND)
                        eng = drain_eng[xg % 2]
                        src = AP(
                            x_t[sx],
                            xg * RUN + xl,
                            [[XPITCH, 48], [WIN_C, ND], [1, ND]],
                        )
                        e = copy_op(eng, cv[:, xl], src)
                        extract_by_sx[sx].append(e)
                        for g in reload_by_sx[(sx, xb)]:
                            add_dep_helper(e.ins, g.ins, reason="X RAW")
                        if xl == XB - 1:
                            nc.sync.dma_start(
                                out=od_t.ap()[sx][:, xb * XB * D : (xb + 1) * XB * D],
                                in_=c_t[xb][:],
                            )

    nc.compile()
    return nc


def _get_program():
    if "nc" not in _CACHE:
        _CACHE["nc"] = _build()
    return _CACHE["nc"]


def kernel(input1: np.ndarray, input2: np.ndarray) -> np.ndarray:
    import ml_dtypes
    from concourse import bass_utils

    nc = _get_program()
    B = input1.shape[0]
    # stage as bf16; fold the exact power-of-two 1/C scale into i1
    i1b = (np.ascontiguousarray(input1, dtype=np.float32) * (1.0 / C)).astype(
        ml_dtypes.bfloat16
    ).reshape(B, C, H * W)
    i2b = np.ascontiguousarray(input2, dtype=np.float32).astype(
        ml_dtypes.bfloat16
    ).reshape(B, C, H * W)
    in_maps = [{"i1": i1b[b], "i2": i2b[b]} for b in range(B)]
    res = bass_utils.run_bass_kernel_spmd(nc, in_maps, core_ids=list(range(B)))
    od = np.stack([np.asarray(r["od"]) for r in res.results])
    od = od.astype(np.float32).reshape(B, 2, 48, SUB_W, D)
    # device row order is q = 16*yb + 8*sy + ry; un-permute to y = 2*ys + sy
    q = np.arange(48)
    yf = 16 * (q // 16) + 2 * (q % 8) + (q % 16) // 8
    inv = np.empty(48, dtype=np.int64)
    inv[yf] = q
    od = od[:, :, inv]
    # [b, sx, y, xg, d] -> [b, d, y, xg, sx] -> [b, d, y, x]
    out = od.transpose(0, 4, 2, 3, 1).reshape(B, D, H, W)
    return np.ascontiguousarray(out)


# revision 33
# speedup vs baseline: 2.2740x; 1.0014x over previous
"""FlowNet correlation kernel for Trainium2 (8 NeuronCores, batch-parallel).

Problem: out[b, d, y, x] = (1/C) * sum_c i1[b,c,y,x] * pad(i2)[b,c,y+dy,x+dx]
  B=8, C=256, H=48, W=64, pad=20, displacements dy,dx in {-20..20 step 2}
  (21x21 = 441), output [8, 441, 48, 64] fp32.

Strategy (per core, one batch element):
  Displacement stride 2 => 4 polyphase subproblems s=(sy,sx), each a dense
  +-10 correlation on a 24x32 quarter image. Inputs are staged to the device
  as bf16 (i1 pre-scaled by 1/C, exact in bf16) so matmuls stream at
  1 col/cycle and all on-chip traffic is half-width.

  Per s, pixel blocks of 8 sub-rows x 16 sub-cols (M=128, block-major
  stationary): all-pairs band matmul against the 28x36 window, clipped to
  in-range window rows/cols (out-of-range correlations are exact zeros kept
  in a pre-zeroed band buffer). PSUM [128,1024] in 2 banks (window rows
  </>= 14), packed 26 columns wide. Drain PSUM -> band bf16 (vector/scalar
  copies, cast on copy).

  The per-pixel displacement extraction avoids both tiny diagonal DMA
  packets (the v0 bottleneck: 64512 x 84B descriptors, ~206us serialized on
  one queue) and SBUF->SBUF remaps (v2 bottleneck: restricted to 8 DMA
  engines at ~7.5 GB/s each):
    1. dump: per-s band [128,6048] -> HBM scratch, 12KB packets, full rate.
    2. reload: HBM -> X[48p, 32 chunks x 756] where the DRAM-side AP embeds
       the per-ry window-row clip (dim-0 stride 16*6048+36); the SBUF dst is
       a plain partition-step-1 scatter. X partition order q=16yb+8sy+ry.
    3. shear: per output column xg, one vector/scalar copy [48p,21,21]
       (free offset xg*756 + xg%16) compacts 441 values into C.
    4. one output DMA per x-parity: C [48, 32*441] bf16 -> HBM, 28KB runs.
  Host upcasts bf16->f32, un-permutes q->y, and transposes to [d,y,x].
"""

import numpy as np

C = 256
H, W = 48, 64
ND = 21          # displacements per axis
D = ND * ND      # 441
SUB_H, SUB_W = H // 2, W // 2      # 24, 32
RB, XB = 8, 16                     # pixel block: 8 sub-rows x 16 sub-cols
WIN_R, WIN_C = RB + 20, XB + 20    # 28 x 36 window per block
BLK = WIN_R * WIN_C                # 1008 band els per block
N_YB, N_XB = SUB_H // RB, SUB_W // XB   # 3, 2
BAND_N = N_YB * N_XB * BLK         # 6048
RUN = ND * WIN_C                   # 756: per-pixel clipped window
XPITCH = SUB_W * RUN               # 24192: X free els
CPITCH = SUB_W * D                 # 14112: C free els

# valid local window rows per yb (real i2 sub-rows are window rows 10..33)
ROWR = [(10, 28), (2, 26), (0, 18)]
# valid local window cols per xb (real i2 sub-cols are window cols 10..41)
COLR = [(10, 36), (0, 26)]
BANK_SPLIT = 14   # window rows < 14 -> psum bank 0, >= 14 -> bank 1

_CACHE = {}


def _build():
    import concourse.bacc as bacc
    import concourse.mybir as mybir
    from concourse.bass_types import AP, SBTensorHandle
    from concourse.tile import TileContext
    from bass_rust import add_dep_helper

    bf16 = mybir.dt.bfloat16

    def alias_sbuf(nc, name, shape, dtype, offset, base_partition):
        # SBUF view at a fixed byte offset and nonzero base partition, so
        # scatter APs keep their start offset inside one partition row
        # (walrus rejects partition-crossing offsets on irregular APs).
        uname = nc._get_name(name, add_next_id=True)
        nc._tensor(uname, list(shape), dtype, type="SB")
        import functools, operator
        per_part = functools.reduce(operator.mul, shape[1:]) * mybir.dt.size(dtype)
        h = SBTensorHandle(
            uname,
            list(shape),
            dtype,
            base_partition=base_partition,
            manual_sbuf_range=(offset, offset + per_part),
            manual_base_name=name,
        )
        mloc = nc.lookup_mloc(h)
        mloc.allocated = True
        mloc.addr = offset
        mloc.base = base_partition
        return h

    nc = bacc.Bacc("TRN2", target_bir_lowering=False, debug=False)
    i1_t = nc.dram_tensor("i1", [C, H * W], bf16, kind="ExternalInput")
    i2_t = nc.dram_tensor("i2", [C, H * W], bf16, kind="ExternalInput")
    od_t = nc.dram_tensor("od", [2, 48, CPITCH], bf16, kind="ExternalOutput")
    # scratch slots padded to 8*RY_STRIDE so the reload's (sy, ry) pair
    # collapses into one 16-count dim-0 (wider SBUF partition span per DMA
    # => more DMA engines participate)
    RY_STRIDE = 16 * BAND_N + WIN_C          # 96804
    SLOT = 8 * RY_STRIDE                      # 774432 (>= 128*BAND_N)
    sc_t = nc.dram_tensor("scr", [2, 2, SLOT], bf16, kind="Internal")

    # raw SBUF tensors accessed with flat/irregular APs
    band = [nc.alloc_sbuf_tensor(f"band{i}", [128, BAND_N], bf16) for i in range(2)]
    x_t = []
    x_alias = []
    for xi in range(2):
        xt = nc.alloc_sbuf_tensor(f"xt{xi}", [48, XPITCH], bf16)
        x_addr = nc.lookup_mloc(xt).addr
        x_t.append(xt)
        x_alias.append({
            bp: alias_sbuf(nc, f"x{xi}al{bp}", [16, XPITCH], bf16, x_addr, bp)
            for bp in (0, 16, 32)
        })

    with TileContext(nc) as tc:
        with (
            tc.tile_pool(name="inp", bufs=1) as inp_pool,
            tc.tile_pool(name="ps", bufs=4, space="PSUM") as ps_pool,
        ):
            stage = [
                inp_pool.tile([128, H * W], bf16, name=f"st{i}", tag=f"st{i}")
                for i in range(2)
            ]
            i1s = [
                [
                    inp_pool.tile([128, SUB_H * SUB_W], bf16, name=f"i1s{k}{s}",
                                  tag=f"i1s{k}{s}")
                    for s in range(4)
                ]
                for k in range(2)
            ]
            i2s = [
                [
                    inp_pool.tile([128, SUB_H * SUB_W], bf16, name=f"i2s{k}{s}",
                                  tag=f"i2s{k}{s}")
                    for s in range(4)
                ]
                for k in range(2)
            ]
            c_t = [
                inp_pool.tile([48, XB * D], bf16, name=f"ct{i}", tag=f"ct{i}")
                for i in range(2)
            ]

            # pre-zero the band buffers once; drains only touch valid
            # window cells, so pad cells stay exactly zero for every s.
            nc.gpsimd.memset(band[0].ap(), 0.0)
            nc.gpsimd.memset(band[1].ap(), 0.0)

            dei_eng = [nc.vector, nc.scalar]
            n_dei = 0

            def copy_op(eng, dst, src):
                if eng is nc.scalar:
                    return eng.copy(dst, src)
                return eng.tensor_copy(dst, src)

            # ---- input staging: i1 and i2 load concurrently into separate
            # stage buffers on separate queues; polyphase on chip. s-order
            # (0, 2, 1, 3) readies subproblem (0,0) slices first.
            for k in range(2):
                cs = slice(128 * k, 128 * (k + 1))
                nc.sync.dma_start(out=stage[0][:], in_=i1_t.ap()[cs])
                nc.scalar.dma_start(out=stage[1][:], in_=i2_t.ap()[cs])
                stv = stage[1][:].rearrange("c (h w) -> c h w", h=H)
                for s in (0, 2, 1, 3):
                    sy, sx = s >> 1, s & 1
                    # block-major i1s: pixel block (yb,xb) contiguous at
                    # offset (2yb+xb)*128 (stationary needs 1 free dim)
                    dstv = i1s[k][s][:].rearrange(
                        "c (b r x) -> c b r x", b=N_YB * N_XB, r=RB
                    )
                    for yb in range(N_YB):
                        src = AP(
                            stage[0][:].tensor,
                            (16 * yb + sy) * W + sx,
                            [[H * W, 128], [2 * XB, N_XB], [2 * W, RB], [2, XB]],
                        )
                        eng = dei_eng[n_dei % 2]
                        n_dei += 1
                        copy_op(eng, dstv[:, 2 * yb : 2 * yb + 2], src)
                    dst = i2s[k][s][:].rearrange("c (h w) -> c h w", h=SUB_H)
                    eng = dei_eng[n_dei % 2]
                    n_dei += 1
                    copy_op(eng, dst, stv[:, sy::2, sx::2])

            i2vv = [
                [i2s[k][s][:].rearrange("c (h w) -> c h w", h=SUB_H) for s in range(4)]
                for k in range(2)
            ]

            # ---- main loop ----
            s_order = [(0, 0), (1, 0), (0, 1), (1, 1)]
            reload_by_sx = {(sx, xb): [] for sx in range(2) for xb in range(2)}
            extract_by_sx = {0: [], 1: []}
            drain_eng = [nc.vector, nc.scalar]
            dma_eng = [nc.sync, nc.scalar]
            n_drain = 0
            n_reload = 0
            for si, (sy, sx) in enumerate(s_order):
                s = 2 * sy + sx
                bnd = band[si % 2]
                bndv = bnd.ap().rearrange(
                    "p (b r c) -> p b r c", b=N_YB * N_XB, r=WIN_R
                )
                for yb in range(N_YB):
                    for xb in range(N_XB):
                        bi = 2 * yb + xb
                        c0, c1 = COLR[xb]
                        nco = c1 - c0
                        ps = ps_pool.tile([128, 1024], mybir.dt.float32, name="ps")
                        for k in range(2):
                            lhs = i1s[k][s][:, 128 * bi : 128 * bi + 128]
                            for bank in range(2):
                                ra, rb_ = ROWR[yb]
                                ra = max(ra, BANK_SPLIT * bank)
                                rb_ = min(rb_, BANK_SPLIT * (bank + 1))
                                if ra >= rb_:
                                    continue
                                rhs = i2vv[k][s][
                                    :,
                                    RB * yb + ra - 10 : RB * yb + rb_ - 10,
                                    XB * xb + c0 - 10 : XB * xb + c1 - 10,
                                ]
                                po = 512 * bank + (ra - BANK_SPLIT * bank) * nco
                                nc.tensor.matmul(
                                    ps[:, po : po + (rb_ - ra) * nco],
                                    lhsT=lhs,
                                    rhs=rhs,
                                    start=(k == 0),
                                    stop=(k == 1),
                                )
                        # drain psum -> band (cast to bf16)
                        for bank in range(2):
                            ra, rb_ = ROWR[yb]
                            ra = max(ra, BANK_SPLIT * bank)
                            rb_ = min(rb_, BANK_SPLIT * (bank + 1))
                            if ra >= rb_:
                                continue
                            po = 512 * bank + (ra - BANK_SPLIT * bank) * nco
                            src = ps[:, po : po + (rb_ - ra) * nco].rearrange(
                                "p (r c) -> p r c", r=rb_ - ra
                            )
                            eng = drain_eng[n_drain % 2]
                            n_drain += 1
                            copy_op(eng, bndv[:, bi, ra:rb_, c0:c1], src)

                    # dump this yb's two blocks to HBM scratch right after
                    # their drains (pipelines the reload into the compute)
                    nc.sync.dma_start(
                        out=AP(
                            sc_t,
                            (sx * 2 + sy) * SLOT + 2 * yb * BLK,
                            [[BAND_N, 128], [1, 2 * BLK]],
                        ),
                        in_=AP(bnd, 2 * yb * BLK, [[BAND_N, 128], [1, 2 * BLK]]),
                    )
                    if si % 2 == 1:
                        # both sy of this (sx, yb) dumped -> reload into
                        # X[sx]. The DRAM-side AP clips each pixel to its 21
                        # window rows; (sy, ry) collapse into one 16-count
                        # dim (slot pitch = 8*RY_STRIDE) so each DMA writes
                        # 16 SBUF partitions. Plain partition-step-1 dst.
                        for xb in range(N_XB):
                            bi = 2 * yb + xb
                            src = AP(
                                sc_t,
                                sx * 2 * SLOT + bi * BLK,
                                [[RY_STRIDE, 16], [BAND_N, XB], [1, RUN]],
                            )
                            dst = AP(
                                x_alias[sx][16 * yb],
                                xb * XB * RUN,
                                [[XPITCH, 16], [RUN, XB], [1, RUN]],
                            )
                            g = dma_eng[n_reload % 2].dma_start(out=dst, in_=src)
                            n_reload += 1
                            reload_by_sx[(sx, xb)].append(g)

                if si % 2 == 1:
                    # extract: per output column, compact 441 of 756
                    for xg in range(SUB_W):
                        xb, xl = xg // XB, xg % XB
                        cv = c_t[xb][:].rearrange("p (x a b) -> p x a b", x=XB, a=ND)
                        eng = drain_eng[xg % 2]
                        src = AP(
                            x_t[sx],
                            xg * RUN + xl,
                            [[XPITCH, 48], [WIN_C, ND], [1, ND]],
                        )
                        e = copy_op(eng, cv[:, xl], src)
                        extract_by_sx[sx].append(e)
                        for g in reload_by_sx[(sx, xb)]:
                            add_dep_helper(e.ins, g.ins, reason="X RAW")
                        if xl == XB - 1:
                            nc.sync.dma_start(
                                out=od_t.ap()[sx][:, xb * XB * D : (xb + 1) * XB * D],
                                in_=c_t[xb][:],
                            )

    nc.compile()
    return nc


def _get_program():
    if "nc" not in _CACHE:
        _CACHE["nc"] = _build()
    return _CACHE["nc"]


def kernel(input1: np.ndarray, input2: np.ndarray) -> np.ndarray:
    import ml_dtypes
    from concourse import bass_utils

    nc = _get_program()
    B = input1.shape[0]
    # stage as bf16; fold the exact power-of-two 1/C scale into i1
    i1b = (np.ascontiguousarray(input1, dtype=np.float32) * (1.0 / C)).astype(
        ml_dtypes.bfloat16
    ).reshape(B, C, H * W)
    i2b = np.ascontiguousarray(input2, dtype=np.float32).astype(
        ml_dtypes.bfloat16
    ).reshape(B, C, H * W)
    in_maps = [{"i1": i1b[b], "i2": i2b[b]} for b in range(B)]
    res = bass_utils.run_bass_kernel_spmd(nc, in_maps, core_ids=list(range(B)))
    od = np.stack([np.asarray(r["od"]) for r in res.results])
    od = od.astype(np.float32).reshape(B, 2, 48, SUB_W, D)
    # device row order is q = 16*yb + 8*sy + ry; un-permute to y = 2*ys + sy
    q = np.arange(48)
    yf = 16 * (q // 16) + 2 * (q % 8) + (q % 16) // 8
    inv = np.empty(48, dtype=np.int64)
    inv[yf] = q
    od = od[:, :, inv]
    # [b, sx, y, xg, d] -> [b, d, y, xg, sx] -> [b, d, y, x]
    out = od.transpose(0, 4, 2, 3, 1).reshape(B, D, H, W)
    return np.ascontiguousarray(out)


# revision 34
# speedup vs baseline: 2.3246x; 1.0222x over previous
"""FlowNet correlation kernel for Trainium2 (8 NeuronCores, batch-parallel).

Problem: out[b, d, y, x] = (1/C) * sum_c i1[b,c,y,x] * pad(i2)[b,c,y+dy,x+dx]
  B=8, C=256, H=48, W=64, pad=20, displacements dy,dx in {-20..20 step 2}
  (21x21 = 441), output [8, 441, 48, 64] fp32.

Strategy (per core, one batch element):
  Displacement stride 2 => 4 polyphase subproblems s=(sy,sx), each a dense
  +-10 correlation on a 24x32 quarter image. Inputs are staged to the device
  as bf16 (i1 pre-scaled by 1/C, exact in bf16) so matmuls stream at
  1 col/cycle and all on-chip traffic is half-width.

  Per s, pixel blocks of 8 sub-rows x 16 sub-cols (M=128, block-major
  stationary): all-pairs band matmul against the 28x36 window, clipped to
  in-range window rows/cols (out-of-range correlations are exact zeros kept
  in a pre-zeroed band buffer). PSUM [128,1024] in 2 banks (window rows
  </>= 14), packed 26 columns wide. Drain PSUM -> band bf16 (vector/scalar
  copies, cast on copy).

  The per-pixel displacement extraction avoids both tiny diagonal DMA
  packets (the v0 bottleneck: 64512 x 84B descriptors, ~206us serialized on
  one queue) and SBUF->SBUF remaps (v2 bottleneck: restricted to 8 DMA
  engines at ~7.5 GB/s each):
    1. dump: per-s band [128,6048] -> HBM scratch, 12KB packets, full rate.
    2. reload: HBM -> X[48p, 32 chunks x 756] where the DRAM-side AP embeds
       the per-ry window-row clip (dim-0 stride 16*6048+36); the SBUF dst is
       a plain partition-step-1 scatter. X partition order q=16yb+8sy+ry.
    3. shear: per output column xg, one vector/scalar copy [48p,21,21]
       (free offset xg*756 + xg%16) compacts 441 values into C.
    4. one output DMA per x-parity: C [48, 32*441] bf16 -> HBM, 28KB runs.
  Host upcasts bf16->f32, un-permutes q->y, and transposes to [d,y,x].
"""

import numpy as np

C = 256
H, W = 48, 64
ND = 21          # displacements per axis
D = ND * ND      # 441
SUB_H, SUB_W = H // 2, W // 2      # 24, 32
RB, XB = 8, 16                     # pixel block: 8 sub-rows x 16 sub-cols
WIN_R, WIN_C = RB + 20, XB + 20    # 28 x 36 window per block
BLK = WIN_R * WIN_C                # 1008 band els per block
N_YB, N_XB = SUB_H // RB, SUB_W // XB   # 3, 2
BAND_N = N_YB * N_XB * BLK         # 6048
RUN = ND * WIN_C                   # 756: per-pixel clipped window
XPITCH = SUB_W * RUN               # 24192: X free els
CPITCH = SUB_W * D                 # 14112: C free els

# valid local window rows per yb (real i2 sub-rows are window rows 10..33)
ROWR = [(10, 28), (2, 26), (0, 18)]
# valid local window cols per xb (real i2 sub-cols are window cols 10..41)
COLR = [(10, 36), (0, 26)]
BANK_SPLIT = 14   # window rows < 14 -> psum bank 0, >= 14 -> bank 1

_CACHE = {}


def _build():
    import concourse.bacc as bacc
    import concourse.mybir as mybir
    from concourse.bass_types import AP, SBTensorHandle
    from concourse.tile import TileContext
    from bass_rust import add_dep_helper

    bf16 = mybir.dt.bfloat16

    def alias_sbuf(nc, name, shape, dtype, offset, base_partition):
        # SBUF view at a fixed byte offset and nonzero base partition, so
        # scatter APs keep their start offset inside one partition row
        # (walrus rejects partition-crossing offsets on irregular APs).
        uname = nc._get_name(name, add_next_id=True)
        nc._tensor(uname, list(shape), dtype, type="SB")
        import functools, operator
        per_part = functools.reduce(operator.mul, shape[1:]) * mybir.dt.size(dtype)
        h = SBTensorHandle(
            uname,
            list(shape),
            dtype,
            base_partition=base_partition,
            manual_sbuf_range=(offset, offset + per_part),
            manual_base_name=name,
        )
        mloc = nc.lookup_mloc(h)
        mloc.allocated = True
        mloc.addr = offset
        mloc.base = base_partition
        return h

    nc = bacc.Bacc("TRN2", target_bir_lowering=False, debug=False)
    i1_t = nc.dram_tensor("i1", [C, H * W], bf16, kind="ExternalInput")
    i2_t = nc.dram_tensor("i2", [C, H * W], bf16, kind="ExternalInput")
    od_t = nc.dram_tensor("od", [2, 48, CPITCH], bf16, kind="ExternalOutput")
    # scratch slots padded to 8*RY_STRIDE so the reload's (sy, ry) pair
    # collapses into one 16-count dim-0 (wider SBUF partition span per DMA
    # => more DMA engines participate)
    RY_STRIDE = 16 * BAND_N + WIN_C          # 96804
    SLOT = 8 * RY_STRIDE                      # 774432 (>= 128*BAND_N)
    sc_t = nc.dram_tensor("scr", [2, 2, SLOT], bf16, kind="Internal")

    # raw SBUF tensors accessed with flat/irregular APs
    band = [nc.alloc_sbuf_tensor(f"band{i}", [128, BAND_N], bf16) for i in range(2)]
    x_t = []
    x_alias = []
    for xi in range(2):
        xt = nc.alloc_sbuf_tensor(f"xt{xi}", [48, XPITCH], bf16)
        x_addr = nc.lookup_mloc(xt).addr
        x_t.append(xt)
        x_alias.append({
            bp: alias_sbuf(nc, f"x{xi}al{bp}", [16, XPITCH], bf16, x_addr, bp)
            for bp in (0, 16, 32)
        })

    with TileContext(nc) as tc:
        with (
            tc.tile_pool(name="inp", bufs=1) as inp_pool,
            tc.tile_pool(name="ps", bufs=4, space="PSUM") as ps_pool,
        ):
            stage = [
                inp_pool.tile([128, H * W], bf16, name=f"st{i}", tag=f"st{i}")
                for i in range(2)
            ]
            i1s = [
                [
                    inp_pool.tile([128, SUB_H * SUB_W], bf16, name=f"i1s{k}{s}",
                                  tag=f"i1s{k}{s}")
                    for s in range(4)
                ]
                for k in range(2)
            ]
            i2s = [
                [
                    inp_pool.tile([128, SUB_H * SUB_W], bf16, name=f"i2s{k}{s}",
                                  tag=f"i2s{k}{s}")
                    for s in range(4)
                ]
                for k in range(2)
            ]
            c_t = [
                inp_pool.tile([48, XB * D], bf16, name=f"ct{i}", tag=f"ct{i}")
                for i in range(2)
            ]

            # pre-zero the band buffers once; drains only touch valid
            # window cells, so pad cells stay exactly zero for every s.
            nc.gpsimd.memset(band[0].ap(), 0.0)
            nc.gpsimd.memset(band[1].ap(), 0.0)

            dei_eng = [nc.vector, nc.scalar]
            n_dei = 0

            def copy_op(eng, dst, src):
                if eng is nc.scalar:
                    return eng.copy(dst, src)
                return eng.tensor_copy(dst, src)

            # ---- input staging: i1 and i2 load concurrently into separate
            # stage buffers on separate queues; polyphase on chip. s-order
            # (0, 2, 1, 3) readies subproblem (0,0) slices first.
            for k in range(2):
                cs = slice(128 * k, 128 * (k + 1))
                nc.sync.dma_start(out=stage[0][:], in_=i1_t.ap()[cs])
                nc.scalar.dma_start(out=stage[1][:], in_=i2_t.ap()[cs])
                stv = stage[1][:].rearrange("c (h w) -> c h w", h=H)
                for s in (0, 2, 1, 3):
                    sy, sx = s >> 1, s & 1
                    # block-major i1s: pixel block (yb,xb) contiguous at
                    # offset (2yb+xb)*128 (stationary needs 1 free dim)
                    dstv = i1s[k][s][:].rearrange(
                        "c (b r x) -> c b r x", b=N_YB * N_XB, r=RB
                    )
                    for yb in range(N_YB):
                        src = AP(
                            stage[0][:].tensor,
                            (16 * yb + sy) * W + sx,
                            [[H * W, 128], [2 * XB, N_XB], [2 * W, RB], [2, XB]],
                        )
                        eng = dei_eng[n_dei % 2]
                        n_dei += 1
                        copy_op(eng, dstv[:, 2 * yb : 2 * yb + 2], src)
                    dst = i2s[k][s][:].rearrange("c (h w) -> c h w", h=SUB_H)
                    eng = dei_eng[n_dei % 2]
                    n_dei += 1
                    copy_op(eng, dst, stv[:, sy::2, sx::2])

            i2vv = [
                [i2s[k][s][:].rearrange("c (h w) -> c h w", h=SUB_H) for s in range(4)]
                for k in range(2)
            ]

            # ---- main loop ----
            s_order = [(0, 0), (1, 0), (0, 1), (1, 1)]
            reload_by_sx = {(sx, xb): [] for sx in range(2) for xb in range(2)}
            extract_by_sx = {0: [], 1: []}
            drain_eng = [nc.vector, nc.scalar]
            dma_eng = [nc.sync, nc.scalar]
            n_drain = 0
            n_reload = 0
            for si, (sy, sx) in enumerate(s_order):
                s = 2 * sy + sx
                bnd = band[si % 2]
                bndv = bnd.ap().rearrange(
                    "p (b r c) -> p b r c", b=N_YB * N_XB, r=WIN_R
                )
                for yb in range(N_YB):
                    for xb in range(N_XB):
                        bi = 2 * yb + xb
                        c0, c1 = COLR[xb]
                        nco = c1 - c0
                        ps = ps_pool.tile([128, 1024], mybir.dt.float32, name="ps")
                        for k in range(2):
                            lhs = i1s[k][s][:, 128 * bi : 128 * bi + 128]
                            for bank in range(2):
                                ra, rb_ = ROWR[yb]
                                ra = max(ra, BANK_SPLIT * bank)
                                rb_ = min(rb_, BANK_SPLIT * (bank + 1))
                                if ra >= rb_:
                                    continue
                                rhs = i2vv[k][s][
                                    :,
                                    RB * yb + ra - 10 : RB * yb + rb_ - 10,
                                    XB * xb + c0 - 10 : XB * xb + c1 - 10,
                                ]
                                po = 512 * bank + (ra - BANK_SPLIT * bank) * nco
                                nc.tensor.matmul(
                                    ps[:, po : po + (rb_ - ra) * nco],
                                    lhsT=lhs,
                                    rhs=rhs,
                                    start=(k == 0),
                                    stop=(k == 1),
                                )
                        # drain psum -> band (cast to bf16)
                        for bank in range(2):
                            ra, rb_ = ROWR[yb]
                            ra = max(ra, BANK_SPLIT * bank)
                            rb_ = min(rb_, BANK_SPLIT * (bank + 1))
                            if ra >= rb_:
                                continue
                            po = 512 * bank + (ra - BANK_SPLIT * bank) * nco
                            src = ps[:, po : po + (rb_ - ra) * nco].rearrange(
                                "p (r c) -> p r c", r=rb_ - ra
                            )
                            eng = drain_eng[n_drain % 2]
                            n_drain += 1
                            copy_op(eng, bndv[:, bi, ra:rb_, c0:c1], src)

                    # dump this yb's two blocks to HBM scratch right after
                    # their drains (pipelines the reload into the compute)
                    nc.sync.dma_start(
                        out=AP(
                            sc_t,
                            (sx * 2 + sy) * SLOT + 2 * yb * BLK,
                            [[BAND_N, 128], [1, 2 * BLK]],
                        ),
                        in_=AP(bnd, 2 * yb * BLK, [[BAND_N, 128], [1, 2 * BLK]]),
                    )
                    if si % 2 == 1:
                        # both sy of this (sx, yb) dumped -> reload into
                        # X[sx]. The DRAM-side AP clips each pixel to its 21
                        # window rows; (sy, ry) collapse into one 16-count
                        # dim (slot pitch = 8*RY_STRIDE) so each DMA writes
                        # 16 SBUF partitions. Plain partition-step-1 dst.
                        for xb in range(N_XB):
                            bi = 2 * yb + xb
                            src = AP(
                                sc_t,
                                sx * 2 * SLOT + bi * BLK,
                                [[RY_STRIDE, 16], [BAND_N, XB], [1, RUN]],
                            )
                            dst = AP(
                                x_alias[sx][16 * yb],
                                xb * XB * RUN,
                                [[XPITCH, 16], [RUN, XB], [1, RUN]],
                            )
                            g = dma_eng[n_reload % 2].dma_start(out=dst, in_=src)
                            n_reload += 1
                            reload_by_sx[(sx, xb)].append(g)

                if si % 2 == 1:
                    # extract: compact 441 of 756 per output column; 4
                    # columns per op (uniform offset delta RUN+1) to cut
                    # instruction overhead on the critical tail
                    for xg0 in range(0, SUB_W, 4):
                        xb, xl0 = xg0 // XB, xg0 % XB
                        cv = c_t[xb][:].rearrange("p (x a b) -> p x a b", x=XB, a=ND)
                        eng = drain_eng[(xg0 // 4) % 2]
                        src = AP(
                            x_t[sx],
                            xg0 * RUN + xl0,
                            [[XPITCH, 48], [RUN + 1, 4], [WIN_C, ND], [1, ND]],
                        )
                        e = copy_op(eng, cv[:, xl0 : xl0 + 4], src)
                        extract_by_sx[sx].append(e)
                        for g in reload_by_sx[(sx, xb)]:
                            add_dep_helper(e.ins, g.ins, reason="X RAW")
                        if xl0 + 4 == XB:
                            nc.sync.dma_start(
                                out=od_t.ap()[sx][:, xb * XB * D : (xb + 1) * XB * D],
                                in_=c_t[xb][:],
                            )

    nc.compile()
    return nc


def _get_program():
    if "nc" not in _CACHE:
        _CACHE["nc"] = _build()
    return _CACHE["nc"]


def kernel(input1: np.ndarray, input2: np.ndarray) -> np.ndarray:
    import ml_dtypes
    from concourse import bass_utils

    nc = _get_program()
    B = input1.shape[0]
    # stage as bf16; fold the exact power-of-two 1/C scale into i1
    i1b = (np.ascontiguousarray(input1, dtype=np.float32) * (1.0 / C)).astype(
        ml_dtypes.bfloat16
    ).reshape(B, C, H * W)
    i2b = np.ascontiguousarray(input2, dtype=np.float32).astype(
        ml_dtypes.bfloat16
    ).reshape(B, C, H * W)
    in_maps = [{"i1": i1b[b], "i2": i2b[b]} for b in range(B)]
    res = bass_utils.run_bass_kernel_spmd(nc, in_maps, core_ids=list(range(B)))
    od = np.stack([np.asarray(r["od"]) for r in res.results])
    od = od.astype(np.float32).reshape(B, 2, 48, SUB_W, D)
    # device row order is q = 16*yb + 8*sy + ry; un-permute to y = 2*ys + sy
    q = np.arange(48)
    yf = 16 * (q // 16) + 2 * (q % 8) + (q % 16) // 8
    inv = np.empty(48, dtype=np.int64)
    inv[yf] = q
    od = od[:, :, inv]
    # [b, sx, y, xg, d] -> [b, d, y, xg, sx] -> [b, d, y, x]
    out = od.transpose(0, 4, 2, 3, 1).reshape(B, D, H, W)
    return np.ascontiguousarray(out)
